# revision 25
# baseline (speedup 1.0000x reference)
"""Trainium2 Bass kernel for nn_Attention_8143257993917.

Multi-head attention (packed QKV + RoPE + additive bias + softmax + head_mask
+ o_proj), B=4, S=2048, D=1024, H=16 heads, fp32 I/O.

Sharding: 8 cores = 4 batches x 2 head-groups (tensor-parallel over heads).
Core c handles batch b = c // 2 and heads g*8..g*8+8 with g = c % 2.
Each core computes a partial output (its heads' contribution through o_proj);
the host sums the two partials per batch and adds o_b.

Device-side design (per core, fast mode):
- Everything runs in "transposed" feature-major layouts so the big score /
  probability matrices never need an on-chip transpose:
    Q_T, K_T: [f, t] (f = head*64+d on partitions): out[f,t] = wT[d,f].T @ hT.
    RoPE: q' = (q + bq) * cos + (rot(q) + rot(bq)) * sin, where the rotated
      branch comes from a SECOND projection with host-prerotated weights
      (rotate_half is a row permutation+sign of W, so it folds into weights).
    V: [t, f] natural layout, so V chunks [k=128, d=64] are directly the
      stationary operand of the PV matmul. A ones-column appended to V makes
      the PV matmul also produce the softmax denominators (row 64 of ctx).
    scores S_T[k, q] = K_T_chunk.T @ Q_T (contraction d=64), fp16 operands,
      fp32 PSUM accumulate.
    bias: exp(S+b) = exp(S)*exp(b); exp(bias) is precomputed on the host in
      fp16 and applied as one elementwise multiply on VectorE (removes 512
      identity-matmul bias adds from the PE).
    exp on ScalarE (PSUM -> SBUF) with a constant -12 shift (softmax is
      shift-invariant; keeps exp outputs inside fp16 range).
    PV is software-pipelined one k-chunk behind scores/exp/mult so the PE
      never waits on the current chunk's ScalarE/VectorE results.
    softmax denominators: exact VectorE reciprocal on a [32, NQH/32] reshape
      (via a small DRAM round-trip on the gpsimd DMA queues, which also
      broadcasts 1/r across 64 partitions); one TT multiply normalizes ctx
      and moves it PSUM -> SBUF.
    head_mask is folded into the V projection weights/bias on the host.
    o_proj: out_T[o, t] = sum_f o_wT[f, o] * ctx_T[f, t], fp16 operands.
  Matmul dtype is fp16 rather than bf16: same PE throughput, ~8x lower
  quantization error (all value ranges verified to fit fp16 comfortably).
  fp32 matmuls on TRN2 lower to LOW_HIGH double-pass + 2 cycles/column
  streaming (~5x slower than fp16), hence the fp16 datapath with fp32
  accumulation; measured end-to-end relative error vs the fp32 reference
  is ~1.3e-3.
"""

import sys

sys.path.insert(0, "/opt/trn_rl_repo")

import numpy as np

_CACHE = {}

H = 16
HPC = 8  # heads per core
G = 2  # head groups


def build_nc(S=2048, D=1024, fast=True):
    """Build + compile the per-core Bass program (same program on all cores)."""
    import concourse.bass as bass
    from concourse import bacc
    import concourse.mybir as mybir
    import concourse.tile as tile
    from concourse.masks import make_identity
    from concourse.tile_rust import add_dep_helper

    F32 = mybir.dt.float32
    BF16 = mybir.dt.bfloat16
    F16 = mybir.dt.float16
    MT = F16 if fast else F32      # matmul operand dtype
    AF = mybir.ActivationFunctionType

    P = 128
    DC = D // P          # d chunks (contraction for projections)
    KC = S // P          # k chunks (scores contraction)
    NQH = S // 2         # q-half size
    NQ = min(512, NQH)   # matmul free-dim chunk
    NQC = NQH // NQ      # chunks per q-half
    FPC = HPC * 64       # features per core (= 512)
    FT = FPC // P        # f-tiles per tensor (= 4)
    NT = min(512, S)     # phase C t-chunk
    TT4 = S // NT
    NTA = min(512, NQH)  # phase A t-chunk

    nc = bacc.Bacc("TRN2", target_bir_lowering=False, debug=False, num_devices=8)

    hT = nc.dram_tensor("hT", [D, S], MT, kind="ExternalInput")
    w4 = nc.dram_tensor("w4", [D, 4 * FPC], MT, kind="ExternalInput")
    b4 = nc.dram_tensor("b4", [4 * FPC], F32, kind="ExternalInput")
    wvT = nc.dram_tensor("wvT", [D, FPC], MT, kind="ExternalInput")
    bv = nc.dram_tensor("bv", [FPC], MT, kind="ExternalInput")
    cosr = nc.dram_tensor("cosr", [P, S], F32, kind="ExternalInput")
    sinr = nc.dram_tensor("sinr", [P, S], F32, kind="ExternalInput")
    if fast:
        expbT = nc.dram_tensor("expbT", [S, S], F16, kind="ExternalInput")
    else:
        biasT = nc.dram_tensor("biasT", [S, S], F32, kind="ExternalInput")
    owT = nc.dram_tensor("owT", [FPC, D], MT, kind="ExternalInput")
    outT = nc.dram_tensor("outT", [D, S], F32, kind="ExternalOutput")

    hT_r = hT.ap().rearrange("(o p) t -> p o t", p=P)
    w4_r = w4.ap().rearrange("(o p) f -> p o f", p=P)
    wv_r = wvT.ap().rearrange("(o p) f -> p o f", p=P)
    ow_r = owT.ap().rearrange("(o p) f -> p o f", p=P)
    b4_r = b4.ap().rearrange("(o p) -> p o", p=P)

    with tile.TileContext(nc) as tc:
        with (
            tc.tile_pool(name="cst", bufs=1) as cst,
            tc.tile_pool(name="pAB", bufs=1) as pAB,
            tc.tile_pool(name="dram", bufs=4, space="DRAM") as dpool,
        ):
            ident = cst.tile([P, P], F32)
            make_identity(nc, ident)
            ones1 = cst.tile([1, P], MT)
            nc.vector.memset(ones1[:], 1.0)
            b4_sb = cst.tile([P, 4 * FPC // P], F32)
            nc.sync.dma_start(b4_sb[:], b4_r)
            bv_sb = cst.tile([1, FPC], MT)
            eshift = cst.tile([P, 1], F32)
            nc.vector.memset(eshift[:], -12.0)
            nc.sync.dma_start(bv_sb[:], bv.ap()[None, :])

            # persistent phase A->B products
            qk_sb = pAB.tile([P, 2 * FT, S], MT)          # slots: Q ft 0..FT-1, K ft FT..2FT-1
            v_sb = pAB.tile([P, KC, HPC, 66], MT)          # col 64 = ones

            nc.vector.memset(v_sb[:, :, :, 64:65], 1.0)

            PSW = max(NQH, 512)  # psum tag width (fp32 elems per partition)

            # ---------------- Phase A: projections + rope ----------------
            with (
                tc.tile_pool(name="pA", bufs=1) as pA,
                tc.tile_pool(name="pAw", bufs=2) as pAw,
                tc.tile_pool(name="psA", bufs=2, space="PSUM") as ppsA,
            ):
                for half in range(2):
                    tsl = slice(half * NQH, (half + 1) * NQH)
                    h_sb = pA.tile([P, DC, NQH], MT, tag="hT")
                    nc.sync.dma_start(h_sb[:], hT_r[:, :, tsl])
                    cos_sb = pA.tile([P, NQH], F32, tag="cos")
                    nc.sync.dma_start(cos_sb[:], cosr.ap()[:, tsl])
                    sin_sb = pA.tile([P, NQH], F32, tag="sin")
                    nc.sync.dma_start(sin_sb[:], sinr.ap()[:, tsl])

                    # Q/K (+rotated twins) -> qk_sb
                    for qk in range(2):            # 0 = Q, 1 = K
                        for ft in range(FT):
                            fcol = qk * 2 * FPC + ft * P       # col of plain tensor in w4
                            frcol = fcol + FPC                 # col of rotated twin
                            wa = pAw.tile([P, DC, P], MT, tag="wA")
                            nc.sync.dma_start(wa[:], w4_r[:, :, fcol:fcol + P])
                            wb = pAw.tile([P, DC, P], MT, tag="wB")
                            nc.sync.dma_start(wb[:], w4_r[:, :, frcol:frcol + P])
                            bcol = (qk * 2 * FPC + ft * P) // P
                            brcol = bcol + FPC // P
                            for tq in range(NQH // NTA):
                                qsl = slice(tq * NTA, (tq + 1) * NTA)
                                pa = ppsA.tile([P, NTA], F32, tag="pa", name="pa")
                                pb = ppsA.tile([P, NTA], F32, tag="pb", name="pb")
                                for dc in range(DC):
                                    nc.tensor.matmul(pa[:], wa[:, dc], h_sb[:, dc, qsl],
                                                     start=(dc == 0), stop=(dc == DC - 1))
                                for dc in range(DC):
                                    nc.tensor.matmul(pb[:], wb[:, dc], h_sb[:, dc, qsl],
                                                     start=(dc == 0), stop=(dc == DC - 1))
                                tca = pAw.tile([P, NTA], F32, tag="tca")
                                nc.vector.scalar_tensor_tensor(
                                    tca[:], pa[:], b4_sb[:, bcol:bcol + 1], cos_sb[:, qsl],
                                    op0=mybir.AluOpType.add, op1=mybir.AluOpType.mult)
                                tcb = pAw.tile([P, NTA], F32, tag="tcb")
                                nc.vector.scalar_tensor_tensor(
                                    tcb[:], pb[:], b4_sb[:, brcol:brcol + 1], sin_sb[:, qsl],
                                    op0=mybir.AluOpType.add, op1=mybir.AluOpType.mult)
                                dst = qk_sb[:, qk * FT + ft, half * NQH + tq * NTA:
                                            half * NQH + (tq + 1) * NTA]
                                nc.vector.tensor_add(dst, tca[:], tcb[:])

                    # V for this half: t-tiles within half
                    wvs = pA.tile([P, DC, FPC], MT, tag="wV")
                    nc.sync.dma_start(wvs[:], wv_r)
                    for tt in range(NQH // P):
                        gt = half * (NQH // P) + tt            # global t-tile = k-chunk
                        pv = ppsA.tile([P, FPC], F32, tag="pv", name="pv")
                        for dc in range(DC):
                            nc.tensor.matmul(pv[:], h_sb[:, dc, tt * P:(tt + 1) * P],
                                             wvs[:, dc], start=(dc == 0), stop=False)
                        nc.tensor.matmul(pv[:], ones1[:], bv_sb[:], start=False, stop=True)
                        nc.vector.tensor_copy(v_sb[:, gt, :, 0:64], pv[:])

            with tc.tile_pool(name="pBC", bufs=1) as pBC:
                ctxT = pBC.tile([P, FT, S], MT)            # normalized ctx, f-major

                # ---------------- Phase B: attention ----------------
                with (
                    tc.tile_pool(name="pB", bufs=2) as pB,
                    tc.tile_pool(name="psB", bufs=1, space="PSUM") as ppsB,
                ):
                    for hp in range(HPC // 2):
                        for qh in range(2):
                            qoff = qh * NQH
                            cps = []
                            for i in range(2):
                                ct = ppsB.tile([P, NQH], F32, tag=f"ctx{i}",
                                               name=f"ctx{i}")
                                cps.append(ct[:65, :])
                            prev_us = None
                            prev_kc = -1
                            for kc in range(KC):
                                if fast:
                                    eb_sb = pB.tile([P, NQH], F16, tag="bias", bufs=3)
                                    nc.sync.dma_start(
                                        eb_sb[:],
                                        expbT.ap()[kc * P:(kc + 1) * P,
                                                   qoff:qoff + NQH])
                                else:
                                    bias_sb = pB.tile([P, NQH], F32, tag="bias")
                                    nc.sync.dma_start(
                                        bias_sb[:],
                                        biasT.ap()[kc * P:(kc + 1) * P,
                                                   qoff:qoff + NQH])
                                psS = []
                                for hi in range(2):
                                    psS.append(ppsB.tile([P, NQH], F32,
                                                         tag=f"s{hi}", name="psS"))
                                # scores: h0/h1 adjacent for row-group overlap
                                prev_mm = None
                                for qc in range(NQC):
                                    csl = slice(qc * NQ, (qc + 1) * NQ)
                                    for hi in range(2):
                                        h = 2 * hp + hi
                                        base = 64 * (h % 2)
                                        ft = h // 2
                                        ksl = qk_sb[base:base + 64, FT + ft,
                                                    kc * P:(kc + 1) * P]
                                        qsl = qk_sb[base:base + 64, ft,
                                                    qoff + qc * NQ:
                                                    qoff + (qc + 1) * NQ]
                                        mm = nc.tensor.matmul(psS[hi][:, csl], ksl,
                                                              qsl, start=True,
                                                              stop=fast)
                                        if prev_mm is not None:
                                            add_dep_helper(
                                                mm.ins, prev_mm.ins, sync=False,
                                                reason="scores row-group pairing")
                                        prev_mm = mm
                                        if not fast:
                                            nc.tensor.matmul(psS[hi][:, csl],
                                                             ident[:],
                                                             bias_sb[:, csl],
                                                             start=False, stop=True)
                                us = []
                                for hi in range(2):
                                    u_sb = pB.tile([P, NQH], MT, tag=f"u{hi}")
                                    if fast:
                                        nc.scalar.activation(u_sb[:], psS[hi][:],
                                                             AF.Exp, bias=eshift[:])
                                        u2 = pB.tile([P, NQH], F16, tag=f"u2{hi}")
                                        nc.vector.tensor_mul(u2[:], u_sb[:],
                                                             eb_sb[:])
                                        us.append(u2)
                                    else:
                                        nc.scalar.activation(u_sb[:], psS[hi][:],
                                                             AF.Exp)
                                        us.append(u_sb)
                                # software-pipeline: PV lags one kc so PE never
                                # waits on this cycle's exp/mult
                                if prev_us is not None:
                                    for qc in range(NQC):
                                        csl = slice(qc * NQ, (qc + 1) * NQ)
                                        for hi in range(2):
                                            h = 2 * hp + hi
                                            nc.tensor.matmul(
                                                cps[hi][:, csl],
                                                v_sb[:, prev_kc, h, 0:65],
                                                prev_us[hi][:, csl],
                                                start=(prev_kc == 0), stop=False)
                                prev_us, prev_kc = us, kc
                            for qc in range(NQC):
                                csl = slice(qc * NQ, (qc + 1) * NQ)
                                for hi in range(2):
                                    h = 2 * hp + hi
                                    nc.tensor.matmul(cps[hi][:, csl],
                                                     v_sb[:, prev_kc, h, 0:65],
                                                     prev_us[hi][:, csl],
                                                     start=False, stop=True)
                            # finalize: stage-interleave both heads' chains to
                            # halve the boundary bubble
                            rts, rscrs, rsqs, rrecs, rscr2s, rbs = [], [], [], [], [], []
                            for hi in range(2):
                                rt = pB.tile([1, NQH], F32, tag=f"rt{hi}")
                                nc.vector.tensor_copy(rt[:], cps[hi][64:65, :])
                                rts.append(rt)
                            for hi in range(2):
                                rscr = dpool.tile([NQH], F32)
                                nc.gpsimd.dma_start(rscr[None, :], rts[hi][:])
                                rscrs.append(rscr)
                            for hi in range(2):
                                rsq = pB.tile([32, NQH // 32], F32, tag=f"rsq{hi}")
                                nc.gpsimd.dma_start(
                                    rsq[:], rscrs[hi].rearrange("(a b) -> a b", a=32))
                                rsqs.append(rsq)
                            for hi in range(2):
                                rrec = pB.tile([32, NQH // 32], F32, tag=f"rrec{hi}")
                                nc.vector.reciprocal(rrec[:], rsqs[hi][:])
                                rrecs.append(rrec)
                            for hi in range(2):
                                rscr2 = dpool.tile([NQH], F32)
                                nc.gpsimd.dma_start(
                                    rscr2.rearrange("(a b) -> a b", a=32), rrecs[hi][:])
                                rscr2s.append(rscr2)
                            for hi in range(2):
                                rb = pB.tile([64, NQH], F32, tag=f"rb{hi}")
                                nc.gpsimd.dma_start(rb[:],
                                                    rscr2s[hi].partition_broadcast(64))
                                rbs.append(rb)
                            for hi in range(2):
                                h = 2 * hp + hi
                                base = 64 * (h % 2)
                                ft = h // 2
                                nc.vector.tensor_mul(
                                    ctxT[base:base + 64, ft, qoff:qoff + NQH],
                                    cps[hi][0:64, :], rbs[hi][:])

                # ---------------- Phase C: output projection ----------------
                with (
                    tc.tile_pool(name="pC", bufs=2) as pC,
                    tc.tile_pool(name="psC", bufs=2, space="PSUM") as ppsC,
                ):
                    ow_sb = pC.tile([P, FT, D], MT, tag="ow")
                    nc.sync.dma_start(ow_sb[:], ow_r)
                    for ot in range(D // P):
                        for tq in range(TT4):
                            tsl = slice(tq * NT, (tq + 1) * NT)
                            po = ppsC.tile([P, NT], F32, tag="po", name="po")
                            for fc in range(FT):
                                nc.tensor.matmul(po[:],
                                                 ow_sb[:, fc, ot * P:(ot + 1) * P],
                                                 ctxT[:, fc, tsl],
                                                 start=(fc == 0), stop=(fc == FT - 1))
                            o_sb = pC.tile([P, NT], F32, tag="oT")
                            nc.scalar.copy(o_sb[:], po[:])
                            nc.sync.dma_start(outT.ap()[ot * P:(ot + 1) * P, tsl],
                                              o_sb[:])

    nc.compile()
    return nc


def make_core_inputs(hidden_states, attention_bias, rope_cos, rope_sin, head_mask,
                     qkv_w, qkv_b, o_w, S=2048, D=1024, fast=True):
    """Host-side sharding + layout preparation. Returns list of 8 input dicts."""
    f32 = np.float32
    mt = np.float16 if fast else np.float32
    f16 = np.float16
    hidden_states = np.asarray(hidden_states, f32)
    attention_bias = np.asarray(attention_bias, f32)
    rope_cos = np.asarray(rope_cos, f32)
    rope_sin = np.asarray(rope_sin, f32)
    head_mask = np.asarray(head_mask, f32).reshape(-1)
    qkv_w = np.asarray(qkv_w, f32)
    qkv_b = np.asarray(qkv_b, f32)
    o_w = np.asarray(o_w, f32)

    B = hidden_states.shape[0]
    FPC = HPC * 64
    F = H * 64  # qkv feature dim (row-section size of qkv_w)

    def rot_rows(w):
        # rows indexed by f = hl*64 + d; rot(q)[d] = -q[d+32] (d<32) else q[d-32]
        w = w.reshape(HPC, 64, -1) if w.ndim == 2 else w.reshape(HPC, 64)
        lo, hi = w[:, 0:32], w[:, 32:64]
        out = np.concatenate([-hi, lo], axis=1)
        return out.reshape(HPC * 64, -1) if out.ndim == 3 else out.reshape(HPC * 64)

    cos_t = rope_cos[0, :, 0, :].T.astype(f32)     # [64, S]
    sin_t = rope_sin[0, :, 0, :].T.astype(f32)
    cosr = np.concatenate([cos_t, cos_t], axis=0)  # [128, S]
    sinr = np.concatenate([sin_t, sin_t], axis=0)

    in_maps = []
    for c in range(8):
        b, g = divmod(c, G)
        fs = slice(g * FPC, (g + 1) * FPC)
        wq = qkv_w[F * 0:F * 1][fs]
        wk = qkv_w[F * 1:F * 2][fs]
        wv = qkv_w[F * 2:F * 3][fs].copy()
        bq = qkv_b[F * 0:F * 1][fs]
        bk = qkv_b[F * 1:F * 2][fs]
        bvv = qkv_b[F * 2:F * 3][fs].copy()
        mask = head_mask[g * HPC:(g + 1) * HPC]
        wv *= np.repeat(mask, 64)[:, None]
        bvv *= np.repeat(mask, 64)
        wqr, bqr = rot_rows(wq), rot_rows(bq)
        wkr, bkr = rot_rows(wk), rot_rows(bk)
        w4 = np.concatenate([wq.T, wqr.T, wk.T, wkr.T], axis=1)  # [D, 4*FPC]
        b4 = np.concatenate([bq, bqr, bk, bkr])
        bT = np.ascontiguousarray(attention_bias[b, 0].T)
        m = {
            "hT": np.ascontiguousarray(hidden_states[b].T).astype(mt),
            "w4": np.ascontiguousarray(w4).astype(mt),
            "b4": np.ascontiguousarray(b4),
            "wvT": np.ascontiguousarray(wv.T).astype(mt),
            "bv": np.ascontiguousarray(bvv).astype(mt),
            "cosr": np.ascontiguousarray(cosr),
            "sinr": np.ascontiguousarray(sinr),
            "owT": np.ascontiguousarray(o_w[:, g * FPC:(g + 1) * FPC].T).astype(mt),
        }
        if fast:
            m["expbT"] = np.exp(bT).astype(f16)
        else:
            m["biasT"] = bT
        in_maps.append(m)
    return in_maps


def kernel(hidden_states, attention_bias, rope_cos, rope_sin, head_mask,
           qkv_w, qkv_b, o_w, o_b, **_unused):
    from concourse.bass_utils import run_bass_kernel_spmd

    B, S, D = hidden_states.shape
    fast = _CACHE.get("fast", True)
    if "nc" not in _CACHE:
        _CACHE["nc"] = build_nc(S=S, D=D, fast=fast)
    nc = _CACHE["nc"]

    in_maps = make_core_inputs(hidden_states, attention_bias, rope_cos, rope_sin,
                               head_mask, qkv_w, qkv_b, o_w, S=S, D=D, fast=fast)
    res = run_bass_kernel_spmd(nc, in_maps, list(range(8)))
    _CACHE["last_results"] = res

    o_b = np.asarray(o_b, np.float32)
    out = np.empty((B, S, D), np.float32)
    for b in range(B):
        acc = res.results[2 * b]["outT"].T + res.results[2 * b + 1]["outT"].T
        out[b] = acc + o_b[None, :]
    return out


# revision 26
# speedup vs baseline: 1.0312x; 1.0312x over previous
"""Trainium2 Bass kernel for nn_Attention_8143257993917.

Multi-head attention (packed QKV + RoPE + additive bias + softmax + head_mask
+ o_proj), B=4, S=2048, D=1024, H=16 heads, fp32 I/O.

Sharding: 8 cores = 4 batches x 2 head-groups (tensor-parallel over heads).
Core c handles batch b = c // 2 and heads g*8..g*8+8 with g = c % 2.
Each core computes a partial output (its heads' contribution through o_proj);
the host sums the two partials per batch and adds o_b.

Device-side design (per core, fast mode):
- Everything runs in "transposed" feature-major layouts so the big score /
  probability matrices never need an on-chip transpose:
    Q_T, K_T: [f, t] (f = head*64+d on partitions): out[f,t] = wT[d,f].T @ hT.
    RoPE: q' = (q + bq) * cos + (rot(q) + rot(bq)) * sin, where the rotated
      branch comes from a SECOND projection with host-prerotated weights
      (rotate_half is a row permutation+sign of W, so it folds into weights).
    V: [t, f] natural layout, so V chunks [k=128, d=64] are directly the
      stationary operand of the PV matmul. A ones-column appended to V makes
      the PV matmul also produce the softmax denominators (row 64 of ctx).
    scores S_T[k, q] = K_T_chunk.T @ Q_T (contraction d=64), fp16 operands,
      fp32 PSUM accumulate.
    bias: exp(S+b) = exp(S)*exp(b); exp(bias) is precomputed on the host in
      fp16 and applied as one elementwise multiply on VectorE (removes 512
      identity-matmul bias adds from the PE).
    exp on ScalarE (PSUM -> SBUF) with a constant -12 shift (softmax is
      shift-invariant; keeps exp outputs inside fp16 range).
    PV is software-pipelined one k-chunk behind scores/exp/mult so the PE
      never waits on the current chunk's ScalarE/VectorE results.
    softmax denominators: exact VectorE reciprocal on a [32, NQH/32] reshape
      (via a small DRAM round-trip on the gpsimd DMA queues, which also
      broadcasts 1/r across 64 partitions); one TT multiply normalizes ctx
      and moves it PSUM -> SBUF.
    head_mask is folded into the V projection weights/bias on the host.
    o_proj: out_T[o, t] = sum_f o_wT[f, o] * ctx_T[f, t], fp16 operands.
  Matmul dtype is fp16 rather than bf16: same PE throughput, ~8x lower
  quantization error (all value ranges verified to fit fp16 comfortably).
  fp32 matmuls on TRN2 lower to LOW_HIGH double-pass + 2 cycles/column
  streaming (~5x slower than fp16), hence the fp16 datapath with fp32
  accumulation; measured end-to-end relative error vs the fp32 reference
  is ~1.3e-3.
"""

import sys

sys.path.insert(0, "/opt/trn_rl_repo")

import numpy as np

_CACHE = {}

H = 16
HPC = 8  # heads per core
G = 2  # head groups


def build_nc(S=2048, D=1024, fast=True):
    """Build + compile the per-core Bass program (same program on all cores)."""
    import concourse.bass as bass
    from concourse import bacc
    import concourse.mybir as mybir
    import concourse.tile as tile
    from concourse.masks import make_identity
    from concourse.tile_rust import add_dep_helper

    F32 = mybir.dt.float32
    BF16 = mybir.dt.bfloat16
    F16 = mybir.dt.float16
    MT = F16 if fast else F32      # matmul operand dtype
    AF = mybir.ActivationFunctionType

    P = 128
    DC = D // P          # d chunks (contraction for projections)
    KC = S // P          # k chunks (scores contraction)
    NQH = S // 2         # q-half size
    NQ = min(512, NQH)   # matmul free-dim chunk
    NQC = NQH // NQ      # chunks per q-half
    FPC = HPC * 64       # features per core (= 512)
    FT = FPC // P        # f-tiles per tensor (= 4)
    NT = min(512, S)     # phase C t-chunk
    TT4 = S // NT
    NTA = min(512, NQH)  # phase A t-chunk

    nc = bacc.Bacc("TRN2", target_bir_lowering=False, debug=False, num_devices=8)

    hT = nc.dram_tensor("hT", [D, S], MT, kind="ExternalInput")
    w4 = nc.dram_tensor("w4", [D, 4 * FPC], MT, kind="ExternalInput")
    b4 = nc.dram_tensor("b4", [4 * FPC], F32, kind="ExternalInput")
    wvT = nc.dram_tensor("wvT", [D, FPC], MT, kind="ExternalInput")
    bv = nc.dram_tensor("bv", [FPC], MT, kind="ExternalInput")
    cosr = nc.dram_tensor("cosr", [P, S], F32, kind="ExternalInput")
    sinr = nc.dram_tensor("sinr", [P, S], F32, kind="ExternalInput")
    if fast:
        expbT = nc.dram_tensor("expbT", [S, S], F16, kind="ExternalInput")
    else:
        biasT = nc.dram_tensor("biasT", [S, S], F32, kind="ExternalInput")
    owT = nc.dram_tensor("owT", [FPC, D], MT, kind="ExternalInput")
    outT = nc.dram_tensor("outT", [D, S], F32, kind="ExternalOutput")

    hT_r = hT.ap().rearrange("(o p) t -> p o t", p=P)
    w4_r = w4.ap().rearrange("(o p) f -> p o f", p=P)
    wv_r = wvT.ap().rearrange("(o p) f -> p o f", p=P)
    ow_r = owT.ap().rearrange("(o p) f -> p o f", p=P)
    b4_r = b4.ap().rearrange("(o p) -> p o", p=P)

    with tile.TileContext(nc) as tc:
        with (
            tc.tile_pool(name="cst", bufs=1) as cst,
            tc.tile_pool(name="pAB", bufs=1) as pAB,
            tc.tile_pool(name="dram", bufs=4, space="DRAM") as dpool,
        ):
            ident = cst.tile([P, P], F32)
            make_identity(nc, ident)
            ones1 = cst.tile([1, P], MT)
            nc.vector.memset(ones1[:], 1.0)
            b4_sb = cst.tile([P, 4 * FPC // P], F32)
            nc.sync.dma_start(b4_sb[:], b4_r)
            bv_sb = cst.tile([1, FPC], MT)
            eshift = cst.tile([P, 1], F32)
            nc.vector.memset(eshift[:], -12.0)
            nc.sync.dma_start(bv_sb[:], bv.ap()[None, :])

            # persistent phase A->B products
            qk_sb = pAB.tile([P, 2 * FT, S], MT)          # slots: Q ft 0..FT-1, K ft FT..2FT-1
            v_sb = pAB.tile([P, KC, HPC, 66], MT)          # col 64 = ones

            nc.vector.memset(v_sb[:, :, :, 64:65], 1.0)

            PSW = max(NQH, 512)  # psum tag width (fp32 elems per partition)

            # ---------------- Phase A: projections + rope ----------------
            with (
                tc.tile_pool(name="pA", bufs=1) as pA,
                tc.tile_pool(name="pAw", bufs=2) as pAw,
                tc.tile_pool(name="psA", bufs=2, space="PSUM") as ppsA,
            ):
                for half in range(2):
                    tsl = slice(half * NQH, (half + 1) * NQH)
                    h_sb = pA.tile([P, DC, NQH], MT, tag="hT", bufs=2)
                    nc.sync.dma_start(h_sb[:], hT_r[:, :, tsl])
                    cos_sb = pA.tile([P, NQH], F32, tag="cos", bufs=2)
                    nc.sync.dma_start(cos_sb[:], cosr.ap()[:, tsl])
                    sin_sb = pA.tile([P, NQH], F32, tag="sin", bufs=2)
                    nc.sync.dma_start(sin_sb[:], sinr.ap()[:, tsl])

                    # Q/K (+rotated twins) -> qk_sb
                    for qk in range(2):            # 0 = Q, 1 = K
                        for ft in range(FT):
                            fcol = qk * 2 * FPC + ft * P       # col of plain tensor in w4
                            frcol = fcol + FPC                 # col of rotated twin
                            wa = pAw.tile([P, DC, P], MT, tag="wA")
                            nc.sync.dma_start(wa[:], w4_r[:, :, fcol:fcol + P])
                            wb = pAw.tile([P, DC, P], MT, tag="wB")
                            nc.sync.dma_start(wb[:], w4_r[:, :, frcol:frcol + P])
                            bcol = (qk * 2 * FPC + ft * P) // P
                            brcol = bcol + FPC // P
                            for tq in range(NQH // NTA):
                                qsl = slice(tq * NTA, (tq + 1) * NTA)
                                pa = ppsA.tile([P, NTA], F32, tag="pa", name="pa")
                                pb = ppsA.tile([P, NTA], F32, tag="pb", name="pb")
                                for dc in range(DC):
                                    nc.tensor.matmul(pa[:], wa[:, dc], h_sb[:, dc, qsl],
                                                     start=(dc == 0), stop=(dc == DC - 1))
                                for dc in range(DC):
                                    nc.tensor.matmul(pb[:], wb[:, dc], h_sb[:, dc, qsl],
                                                     start=(dc == 0), stop=(dc == DC - 1))
                                tca = pAw.tile([P, NTA], F32, tag="tca")
                                nc.vector.scalar_tensor_tensor(
                                    tca[:], pa[:], b4_sb[:, bcol:bcol + 1], cos_sb[:, qsl],
                                    op0=mybir.AluOpType.add, op1=mybir.AluOpType.mult)
                                tcb = pAw.tile([P, NTA], F32, tag="tcb")
                                nc.vector.scalar_tensor_tensor(
                                    tcb[:], pb[:], b4_sb[:, brcol:brcol + 1], sin_sb[:, qsl],
                                    op0=mybir.AluOpType.add, op1=mybir.AluOpType.mult)
                                dst = qk_sb[:, qk * FT + ft, half * NQH + tq * NTA:
                                            half * NQH + (tq + 1) * NTA]
                                nc.vector.tensor_add(dst, tca[:], tcb[:])

                    # V for this half: t-tiles within half
                    wvs = pA.tile([P, DC, FPC], MT, tag="wV")
                    nc.sync.dma_start(wvs[:], wv_r)
                    for tt in range(NQH // P):
                        gt = half * (NQH // P) + tt            # global t-tile = k-chunk
                        pv = ppsA.tile([P, FPC], F32, tag="pv", name="pv")
                        for dc in range(DC):
                            nc.tensor.matmul(pv[:], h_sb[:, dc, tt * P:(tt + 1) * P],
                                             wvs[:, dc], start=(dc == 0), stop=False)
                        nc.tensor.matmul(pv[:], ones1[:], bv_sb[:], start=False, stop=True)
                        nc.vector.tensor_copy(v_sb[:, gt, :, 0:64], pv[:])

            with tc.tile_pool(name="pBC", bufs=1) as pBC:
                ctxT = pBC.tile([P, FT, S], MT)            # normalized ctx, f-major
                ow_sb = pBC.tile([P, FT, D], MT)
                nc.sync.dma_start(ow_sb[:], ow_r)

                # ---------------- Phase B: attention ----------------
                with (
                    tc.tile_pool(name="pB", bufs=2) as pB,
                    tc.tile_pool(name="psB", bufs=1, space="PSUM") as ppsB,
                ):
                    for hp in range(HPC // 2):
                        for qh in range(2):
                            qoff = qh * NQH
                            cps = []
                            for i in range(2):
                                ct = ppsB.tile([P, NQH], F32, tag=f"ctx{i}",
                                               name=f"ctx{i}")
                                cps.append(ct[:65, :])
                            prev_us = None
                            prev_kc = -1
                            for kc in range(KC):
                                if fast:
                                    eb_sb = pB.tile([P, NQH], F16, tag="bias", bufs=3)
                                    nc.sync.dma_start(
                                        eb_sb[:],
                                        expbT.ap()[kc * P:(kc + 1) * P,
                                                   qoff:qoff + NQH])
                                else:
                                    bias_sb = pB.tile([P, NQH], F32, tag="bias")
                                    nc.sync.dma_start(
                                        bias_sb[:],
                                        biasT.ap()[kc * P:(kc + 1) * P,
                                                   qoff:qoff + NQH])
                                psS = []
                                for hi in range(2):
                                    psS.append(ppsB.tile([P, NQH], F32,
                                                         tag=f"s{hi}", name="psS"))
                                # scores: h0/h1 adjacent for row-group overlap
                                prev_mm = None
                                for qc in range(NQC):
                                    csl = slice(qc * NQ, (qc + 1) * NQ)
                                    for hi in range(2):
                                        h = 2 * hp + hi
                                        base = 64 * (h % 2)
                                        ft = h // 2
                                        ksl = qk_sb[base:base + 64, FT + ft,
                                                    kc * P:(kc + 1) * P]
                                        qsl = qk_sb[base:base + 64, ft,
                                                    qoff + qc * NQ:
                                                    qoff + (qc + 1) * NQ]
                                        mm = nc.tensor.matmul(psS[hi][:, csl], ksl,
                                                              qsl, start=True,
                                                              stop=fast)
                                        if prev_mm is not None:
                                            add_dep_helper(
                                                mm.ins, prev_mm.ins, sync=False,
                                                reason="scores row-group pairing")
                                        prev_mm = mm
                                        if not fast:
                                            nc.tensor.matmul(psS[hi][:, csl],
                                                             ident[:],
                                                             bias_sb[:, csl],
                                                             start=False, stop=True)
                                us = []
                                for hi in range(2):
                                    u_sb = pB.tile([P, NQH], MT, tag=f"u{hi}")
                                    if fast:
                                        nc.scalar.activation(u_sb[:], psS[hi][:],
                                                             AF.Exp, bias=eshift[:])
                                        u2 = pB.tile([P, NQH], F16, tag=f"u2{hi}")
                                        nc.vector.tensor_mul(u2[:], u_sb[:],
                                                             eb_sb[:])
                                        us.append(u2)
                                    else:
                                        nc.scalar.activation(u_sb[:], psS[hi][:],
                                                             AF.Exp)
                                        us.append(u_sb)
                                # software-pipeline: PV lags one kc so PE never
                                # waits on this cycle's exp/mult
                                if prev_us is not None:
                                    for qc in range(NQC):
                                        csl = slice(qc * NQ, (qc + 1) * NQ)
                                        for hi in range(2):
                                            h = 2 * hp + hi
                                            nc.tensor.matmul(
                                                cps[hi][:, csl],
                                                v_sb[:, prev_kc, h, 0:65],
                                                prev_us[hi][:, csl],
                                                start=(prev_kc == 0), stop=False)
                                prev_us, prev_kc = us, kc
                            for qc in range(NQC):
                                csl = slice(qc * NQ, (qc + 1) * NQ)
                                for hi in range(2):
                                    h = 2 * hp + hi
                                    nc.tensor.matmul(cps[hi][:, csl],
                                                     v_sb[:, prev_kc, h, 0:65],
                                                     prev_us[hi][:, csl],
                                                     start=False, stop=True)
                            # finalize: stage-interleave both heads' chains to
                            # halve the boundary bubble
                            rts, rscrs, rsqs, rrecs, rscr2s, rbs = [], [], [], [], [], []
                            for hi in range(2):
                                rt = pB.tile([1, NQH], F32, tag=f"rt{hi}")
                                nc.vector.tensor_copy(rt[:], cps[hi][64:65, :])
                                rts.append(rt)
                            for hi in range(2):
                                rscr = dpool.tile([NQH], F32)
                                nc.gpsimd.dma_start(rscr[None, :], rts[hi][:])
                                rscrs.append(rscr)
                            for hi in range(2):
                                rsq = pB.tile([32, NQH // 32], F32, tag=f"rsq{hi}")
                                nc.gpsimd.dma_start(
                                    rsq[:], rscrs[hi].rearrange("(a b) -> a b", a=32))
                                rsqs.append(rsq)
                            for hi in range(2):
                                rrec = pB.tile([32, NQH // 32], F32, tag=f"rrec{hi}")
                                nc.vector.reciprocal(rrec[:], rsqs[hi][:])
                                rrecs.append(rrec)
                            for hi in range(2):
                                rscr2 = dpool.tile([NQH], F32)
                                nc.gpsimd.dma_start(
                                    rscr2.rearrange("(a b) -> a b", a=32), rrecs[hi][:])
                                rscr2s.append(rscr2)
                            for hi in range(2):
                                rb = pB.tile([64, NQH], F32, tag=f"rb{hi}")
                                nc.gpsimd.dma_start(rb[:],
                                                    rscr2s[hi].partition_broadcast(64))
                                rbs.append(rb)
                            for hi in range(2):
                                h = 2 * hp + hi
                                base = 64 * (h % 2)
                                ft = h // 2
                                nc.vector.tensor_mul(
                                    ctxT[base:base + 64, ft, qoff:qoff + NQH],
                                    cps[hi][0:64, :], rbs[hi][:])

                # ---------------- Phase C: output projection ----------------
                with (
                    tc.tile_pool(name="pC", bufs=2) as pC,
                    tc.tile_pool(name="psC", bufs=2, space="PSUM") as ppsC,
                ):
                    for ot in range(D // P):
                        for tq in range(TT4):
                            tsl = slice(tq * NT, (tq + 1) * NT)
                            po = ppsC.tile([P, NT], F32, tag="po", name="po")
                            for fc in range(FT):
                                nc.tensor.matmul(po[:],
                                                 ow_sb[:, fc, ot * P:(ot + 1) * P],
                                                 ctxT[:, fc, tsl],
                                                 start=(fc == 0), stop=(fc == FT - 1))
                            o_sb = pC.tile([P, NT], F32, tag="oT")
                            nc.scalar.copy(o_sb[:], po[:])
                            nc.sync.dma_start(outT.ap()[ot * P:(ot + 1) * P, tsl],
                                              o_sb[:])

    nc.compile()
    return nc


def make_core_inputs(hidden_states, attention_bias, rope_cos, rope_sin, head_mask,
                     qkv_w, qkv_b, o_w, S=2048, D=1024, fast=True):
    """Host-side sharding + layout preparation. Returns list of 8 input dicts."""
    f32 = np.float32
    mt = np.float16 if fast else np.float32
    f16 = np.float16
    hidden_states = np.asarray(hidden_states, f32)
    attention_bias = np.asarray(attention_bias, f32)
    rope_cos = np.asarray(rope_cos, f32)
    rope_sin = np.asarray(rope_sin, f32)
    head_mask = np.asarray(head_mask, f32).reshape(-1)
    qkv_w = np.asarray(qkv_w, f32)
    qkv_b = np.asarray(qkv_b, f32)
    o_w = np.asarray(o_w, f32)

    B = hidden_states.shape[0]
    FPC = HPC * 64
    F = H * 64  # qkv feature dim (row-section size of qkv_w)

    def rot_rows(w):
        # rows indexed by f = hl*64 + d; rot(q)[d] = -q[d+32] (d<32) else q[d-32]
        w = w.reshape(HPC, 64, -1) if w.ndim == 2 else w.reshape(HPC, 64)
        lo, hi = w[:, 0:32], w[:, 32:64]
        out = np.concatenate([-hi, lo], axis=1)
        return out.reshape(HPC * 64, -1) if out.ndim == 3 else out.reshape(HPC * 64)

    cos_t = rope_cos[0, :, 0, :].T.astype(f32)     # [64, S]
    sin_t = rope_sin[0, :, 0, :].T.astype(f32)
    cosr = np.concatenate([cos_t, cos_t], axis=0)  # [128, S]
    sinr = np.concatenate([sin_t, sin_t], axis=0)

    in_maps = []
    for c in range(8):
        b, g = divmod(c, G)
        fs = slice(g * FPC, (g + 1) * FPC)
        wq = qkv_w[F * 0:F * 1][fs]
        wk = qkv_w[F * 1:F * 2][fs]
        wv = qkv_w[F * 2:F * 3][fs].copy()
        bq = qkv_b[F * 0:F * 1][fs]
        bk = qkv_b[F * 1:F * 2][fs]
        bvv = qkv_b[F * 2:F * 3][fs].copy()
        mask = head_mask[g * HPC:(g + 1) * HPC]
        wv *= np.repeat(mask, 64)[:, None]
        bvv *= np.repeat(mask, 64)
        wqr, bqr = rot_rows(wq), rot_rows(bq)
        wkr, bkr = rot_rows(wk), rot_rows(bk)
        w4 = np.concatenate([wq.T, wqr.T, wk.T, wkr.T], axis=1)  # [D, 4*FPC]
        b4 = np.concatenate([bq, bqr, bk, bkr])
        bT = np.ascontiguousarray(attention_bias[b, 0].T)
        m = {
            "hT": np.ascontiguousarray(hidden_states[b].T).astype(mt),
            "w4": np.ascontiguousarray(w4).astype(mt),
            "b4": np.ascontiguousarray(b4),
            "wvT": np.ascontiguousarray(wv.T).astype(mt),
            "bv": np.ascontiguousarray(bvv).astype(mt),
            "cosr": np.ascontiguousarray(cosr),
            "sinr": np.ascontiguousarray(sinr),
            "owT": np.ascontiguousarray(o_w[:, g * FPC:(g + 1) * FPC].T).astype(mt),
        }
        if fast:
            m["expbT"] = np.exp(bT).astype(f16)
        else:
            m["biasT"] = bT
        in_maps.append(m)
    return in_maps


def kernel(hidden_states, attention_bias, rope_cos, rope_sin, head_mask,
           qkv_w, qkv_b, o_w, o_b, **_unused):
    from concourse.bass_utils import run_bass_kernel_spmd

    B, S, D = hidden_states.shape
    fast = _CACHE.get("fast", True)
    if "nc" not in _CACHE:
        _CACHE["nc"] = build_nc(S=S, D=D, fast=fast)
    nc = _CACHE["nc"]

    in_maps = make_core_inputs(hidden_states, attention_bias, rope_cos, rope_sin,
                               head_mask, qkv_w, qkv_b, o_w, S=S, D=D, fast=fast)
    res = run_bass_kernel_spmd(nc, in_maps, list(range(8)))
    _CACHE["last_results"] = res

    o_b = np.asarray(o_b, np.float32)
    out = np.empty((B, S, D), np.float32)
    for b in range(B):
        acc = res.results[2 * b]["outT"].T + res.results[2 * b + 1]["outT"].T
        out[b] = acc + o_b[None, :]
    return out


# revision 27
# speedup vs baseline: 1.0346x; 1.0033x over previous
"""Trainium2 Bass kernel for nn_Attention_8143257993917.

Multi-head attention (packed QKV + RoPE + additive bias + softmax + head_mask
+ o_proj), B=4, S=2048, D=1024, H=16 heads, fp32 I/O.

Sharding: 8 cores = 4 batches x 2 head-groups (tensor-parallel over heads).
Core c handles batch b = c // 2 and heads g*8..g*8+8 with g = c % 2.
Each core computes a partial output (its heads' contribution through o_proj);
the host sums the two partials per batch and adds o_b.

Device-side design (per core, fast mode):
- Everything runs in "transposed" feature-major layouts so the big score /
  probability matrices never need an on-chip transpose:
    Q_T, K_T: [f, t] (f = head*64+d on partitions): out[f,t] = wT[d,f].T @ hT.
    RoPE: q' = (q + bq) * cos + (rot(q) + rot(bq)) * sin, where the rotated
      branch comes from a SECOND projection with host-prerotated weights
      (rotate_half is a row permutation+sign of W, so it folds into weights).
    V: [t, f] natural layout, so V chunks [k=128, d=64] are directly the
      stationary operand of the PV matmul. A ones-column appended to V makes
      the PV matmul also produce the softmax denominators (row 64 of ctx).
    scores S_T[k, q] = K_T_chunk.T @ Q_T (contraction d=64), fp16 operands,
      fp32 PSUM accumulate.
    bias: exp(S+b) = exp(S)*exp(b); exp(bias) is precomputed on the host in
      fp16 and applied as one elementwise multiply on VectorE (removes 512
      identity-matmul bias adds from the PE).
    exp on ScalarE (PSUM -> SBUF) with a constant -12 shift (softmax is
      shift-invariant; keeps exp outputs inside fp16 range).
    PV is software-pipelined one k-chunk behind scores/exp/mult so the PE
      never waits on the current chunk's ScalarE/VectorE results.
    softmax denominators: exact VectorE reciprocal on a [32, NQH/32] reshape
      (via a small DRAM round-trip on the gpsimd DMA queues, which also
      broadcasts 1/r across 64 partitions); one TT multiply normalizes ctx
      and moves it PSUM -> SBUF.
    head_mask is folded into the V projection weights/bias on the host.
    o_proj: out_T[o, t] = sum_f o_wT[f, o] * ctx_T[f, t], fp16 operands.
  Matmul dtype is fp16 rather than bf16: same PE throughput, ~8x lower
  quantization error (all value ranges verified to fit fp16 comfortably).
  fp32 matmuls on TRN2 lower to LOW_HIGH double-pass + 2 cycles/column
  streaming (~5x slower than fp16), hence the fp16 datapath with fp32
  accumulation; measured end-to-end relative error vs the fp32 reference
  is ~1.3e-3.
"""

import sys

sys.path.insert(0, "/opt/trn_rl_repo")

import numpy as np

_CACHE = {}

H = 16
HPC = 8  # heads per core
G = 2  # head groups


def build_nc(S=2048, D=1024, fast=True):
    """Build + compile the per-core Bass program (same program on all cores)."""
    import concourse.bass as bass
    from concourse import bacc
    import concourse.mybir as mybir
    import concourse.tile as tile
    from concourse.masks import make_identity
    from concourse.tile_rust import add_dep_helper

    F32 = mybir.dt.float32
    BF16 = mybir.dt.bfloat16
    F16 = mybir.dt.float16
    MT = F16 if fast else F32      # matmul operand dtype
    AF = mybir.ActivationFunctionType

    P = 128
    DC = D // P          # d chunks (contraction for projections)
    KC = S // P          # k chunks (scores contraction)
    NQH = S // 2         # q-half size
    NQ = min(512, NQH)   # matmul free-dim chunk
    NQC = NQH // NQ      # chunks per q-half
    FPC = HPC * 64       # features per core (= 512)
    FT = FPC // P        # f-tiles per tensor (= 4)
    NT = min(512, S)     # phase C t-chunk
    TT4 = S // NT
    NTA = min(512, NQH)  # phase A t-chunk

    nc = bacc.Bacc("TRN2", target_bir_lowering=False, debug=False, num_devices=8)

    hT = nc.dram_tensor("hT", [D, S], MT, kind="ExternalInput")
    w4 = nc.dram_tensor("w4", [D, 4 * FPC], MT, kind="ExternalInput")
    b4 = nc.dram_tensor("b4", [4 * FPC], F32, kind="ExternalInput")
    wvT = nc.dram_tensor("wvT", [D, FPC], MT, kind="ExternalInput")
    bv = nc.dram_tensor("bv", [FPC], MT, kind="ExternalInput")
    cosr = nc.dram_tensor("cosr", [P, S], F32, kind="ExternalInput")
    sinr = nc.dram_tensor("sinr", [P, S], F32, kind="ExternalInput")
    if fast:
        expbT = nc.dram_tensor("expbT", [S, S], F16, kind="ExternalInput")
    else:
        biasT = nc.dram_tensor("biasT", [S, S], F32, kind="ExternalInput")
    owT = nc.dram_tensor("owT", [FPC, D], MT, kind="ExternalInput")
    outT = nc.dram_tensor("outT", [D, S], F32, kind="ExternalOutput")

    hT_r = hT.ap().rearrange("(o p) t -> p o t", p=P)
    w4_r = w4.ap().rearrange("(o p) f -> p o f", p=P)
    wv_r = wvT.ap().rearrange("(o p) f -> p o f", p=P)
    ow_r = owT.ap().rearrange("(o p) f -> p o f", p=P)
    b4_r = b4.ap().rearrange("(o p) -> p o", p=P)

    with tile.TileContext(nc) as tc:
        with (
            tc.tile_pool(name="cst", bufs=1) as cst,
            tc.tile_pool(name="pAB", bufs=1) as pAB,
            tc.tile_pool(name="dram", bufs=4, space="DRAM") as dpool,
        ):
            ident = cst.tile([P, P], F32)
            make_identity(nc, ident)
            ones1 = cst.tile([1, P], MT)
            nc.vector.memset(ones1[:], 1.0)
            b4_sb = cst.tile([P, 4 * FPC // P], F32)
            nc.sync.dma_start(b4_sb[:], b4_r)
            bv_sb = cst.tile([1, FPC], MT)
            eshift = cst.tile([P, 1], F32)
            nc.vector.memset(eshift[:], -12.0)
            nc.sync.dma_start(bv_sb[:], bv.ap()[None, :])

            # persistent phase A->B products
            qk_sb = pAB.tile([P, 2 * FT, S], MT)          # slots: Q ft 0..FT-1, K ft FT..2FT-1
            v_sb = pAB.tile([P, KC, HPC, 66], MT)          # col 64 = ones

            nc.vector.memset(v_sb[:, :, :, 64:65], 1.0)

            PSW = max(NQH, 512)  # psum tag width (fp32 elems per partition)

            # ---------------- Phase A: projections + rope ----------------
            with (
                tc.tile_pool(name="pA", bufs=1) as pA,
                tc.tile_pool(name="pAw", bufs=2) as pAw,
                tc.tile_pool(name="psA", bufs=2, space="PSUM") as ppsA,
            ):
                for half in range(2):
                    tsl = slice(half * NQH, (half + 1) * NQH)
                    h_sb = pA.tile([P, DC, NQH], MT, tag="hT", bufs=2)
                    nc.sync.dma_start(h_sb[:], hT_r[:, :, tsl])
                    cos_sb = pA.tile([P, NQH], F32, tag="cos", bufs=2)
                    nc.sync.dma_start(cos_sb[:], cosr.ap()[:, tsl])
                    sin_sb = pA.tile([P, NQH], F32, tag="sin", bufs=2)
                    nc.sync.dma_start(sin_sb[:], sinr.ap()[:, tsl])

                    # Q/K (+rotated twins) -> qk_sb
                    for qk in range(2):            # 0 = Q, 1 = K
                        for ft in range(FT):
                            fcol = qk * 2 * FPC + ft * P       # col of plain tensor in w4
                            frcol = fcol + FPC                 # col of rotated twin
                            wa = pAw.tile([P, DC, P], MT, tag="wA")
                            nc.sync.dma_start(wa[:], w4_r[:, :, fcol:fcol + P])
                            wb = pAw.tile([P, DC, P], MT, tag="wB")
                            nc.sync.dma_start(wb[:], w4_r[:, :, frcol:frcol + P])
                            bcol = (qk * 2 * FPC + ft * P) // P
                            brcol = bcol + FPC // P
                            for tq in range(NQH // NTA):
                                qsl = slice(tq * NTA, (tq + 1) * NTA)
                                pa = ppsA.tile([P, NTA], F32, tag="pa", name="pa")
                                pb = ppsA.tile([P, NTA], F32, tag="pb", name="pb")
                                for dc in range(DC):
                                    nc.tensor.matmul(pa[:], wa[:, dc], h_sb[:, dc, qsl],
                                                     start=(dc == 0), stop=(dc == DC - 1))
                                for dc in range(DC):
                                    nc.tensor.matmul(pb[:], wb[:, dc], h_sb[:, dc, qsl],
                                                     start=(dc == 0), stop=(dc == DC - 1))
                                tca = pAw.tile([P, NTA], F32, tag="tca")
                                nc.vector.scalar_tensor_tensor(
                                    tca[:], pa[:], b4_sb[:, bcol:bcol + 1], cos_sb[:, qsl],
                                    op0=mybir.AluOpType.add, op1=mybir.AluOpType.mult)
                                tcb = pAw.tile([P, NTA], F32, tag="tcb")
                                nc.vector.scalar_tensor_tensor(
                                    tcb[:], pb[:], b4_sb[:, brcol:brcol + 1], sin_sb[:, qsl],
                                    op0=mybir.AluOpType.add, op1=mybir.AluOpType.mult)
                                dst = qk_sb[:, qk * FT + ft, half * NQH + tq * NTA:
                                            half * NQH + (tq + 1) * NTA]
                                nc.vector.tensor_add(dst, tca[:], tcb[:])

                    # V for this half: t-tiles within half
                    wvs = pA.tile([P, DC, FPC], MT, tag="wV")
                    nc.sync.dma_start(wvs[:], wv_r)
                    for tt in range(NQH // P):
                        gt = half * (NQH // P) + tt            # global t-tile = k-chunk
                        pv = ppsA.tile([P, FPC], F32, tag="pv", name="pv")
                        for dc in range(DC):
                            nc.tensor.matmul(pv[:], h_sb[:, dc, tt * P:(tt + 1) * P],
                                             wvs[:, dc], start=(dc == 0), stop=False)
                        nc.tensor.matmul(pv[:], ones1[:], bv_sb[:], start=False, stop=True)
                        nc.vector.tensor_copy(v_sb[:, gt, :, 0:64], pv[:])

            with tc.tile_pool(name="pBC", bufs=1) as pBC:
                ctxT = pBC.tile([P, FT, S], MT)            # normalized ctx, f-major
                ow_sb = pBC.tile([P, FT, D], MT)
                nc.sync.dma_start(ow_sb[:], ow_r)

                # ---------------- Phase B: attention ----------------
                with (
                    tc.tile_pool(name="pB", bufs=2) as pB,
                    tc.tile_pool(name="psB", bufs=1, space="PSUM") as ppsB,
                ):
                    for hp in range(HPC // 2):
                        for qh in range(2):
                            qoff = qh * NQH
                            cps = []
                            for i in range(2):
                                ct = ppsB.tile([P, NQH], F32, tag=f"ctx{i}",
                                               name=f"ctx{i}")
                                cps.append(ct[:65, :])
                            prev_us = None
                            prev_kc = -1
                            for kc in range(KC):
                                if fast:
                                    eb_sb = pB.tile([P, NQH], F16, tag="bias", bufs=3)
                                    nc.sync.dma_start(
                                        eb_sb[:],
                                        expbT.ap()[kc * P:(kc + 1) * P,
                                                   qoff:qoff + NQH])
                                else:
                                    bias_sb = pB.tile([P, NQH], F32, tag="bias")
                                    nc.sync.dma_start(
                                        bias_sb[:],
                                        biasT.ap()[kc * P:(kc + 1) * P,
                                                   qoff:qoff + NQH])
                                psS = []
                                for hi in range(2):
                                    psS.append(ppsB.tile([P, NQH], F32,
                                                         tag=f"s{hi}", name="psS"))
                                # scores: h0/h1 adjacent for row-group overlap
                                prev_mm = None
                                for qc in range(NQC):
                                    csl = slice(qc * NQ, (qc + 1) * NQ)
                                    for hi in range(2):
                                        h = 2 * hp + hi
                                        base = 64 * (h % 2)
                                        ft = h // 2
                                        ksl = qk_sb[base:base + 64, FT + ft,
                                                    kc * P:(kc + 1) * P]
                                        qsl = qk_sb[base:base + 64, ft,
                                                    qoff + qc * NQ:
                                                    qoff + (qc + 1) * NQ]
                                        mm = nc.tensor.matmul(psS[hi][:, csl], ksl,
                                                              qsl, start=True,
                                                              stop=fast)
                                        if prev_mm is not None:
                                            add_dep_helper(
                                                mm.ins, prev_mm.ins, sync=False,
                                                reason="scores row-group pairing")
                                        prev_mm = mm
                                        if not fast:
                                            nc.tensor.matmul(psS[hi][:, csl],
                                                             ident[:],
                                                             bias_sb[:, csl],
                                                             start=False, stop=True)
                                us = []
                                for hi in range(2):
                                    u_sb = pB.tile([P, NQH], MT, tag=f"u{hi}")
                                    if fast:
                                        nc.scalar.activation(u_sb[:], psS[hi][:],
                                                             AF.Exp, bias=eshift[:])
                                        u2 = pB.tile([P, NQH], F16, tag=f"u2{hi}")
                                        nc.vector.tensor_mul(u2[:], u_sb[:],
                                                             eb_sb[:])
                                        us.append(u2)
                                    else:
                                        nc.scalar.activation(u_sb[:], psS[hi][:],
                                                             AF.Exp)
                                        us.append(u_sb)
                                # software-pipeline: PV lags one kc so PE never
                                # waits on this cycle's exp/mult
                                if prev_us is not None:
                                    for qc in range(NQC):
                                        csl = slice(qc * NQ, (qc + 1) * NQ)
                                        for hi in range(2):
                                            h = 2 * hp + hi
                                            nc.tensor.matmul(
                                                cps[hi][:, csl],
                                                v_sb[:, prev_kc, h, 0:65],
                                                prev_us[hi][:, csl],
                                                start=(prev_kc == 0), stop=False)
                                prev_us, prev_kc = us, kc
                            for qc in range(NQC):
                                csl = slice(qc * NQ, (qc + 1) * NQ)
                                for hi in range(2):
                                    h = 2 * hp + hi
                                    nc.tensor.matmul(cps[hi][:, csl],
                                                     v_sb[:, prev_kc, h, 0:65],
                                                     prev_us[hi][:, csl],
                                                     start=False, stop=True)
                            # finalize: evacuate ctx PSUM -> SBUF immediately
                            # (frees the ctx banks for the next iteration's PV
                            # without waiting for the reciprocal chain), then
                            # normalize entirely from SBUF.
                            cus = []
                            for hi in range(2):
                                cu = pB.tile([65, NQH], F32, tag=f"cu{hi}")
                                nc.scalar.copy(cu[:], cps[hi][:])
                                cus.append(cu)
                            rscrs, rsqs, rrecs, rscr2s, rbs = [], [], [], [], []
                            for hi in range(2):
                                rscr = dpool.tile([NQH], F32)
                                nc.gpsimd.dma_start(rscr[None, :],
                                                    cus[hi][64:65, :])
                                rscrs.append(rscr)
                            for hi in range(2):
                                rsq = pB.tile([32, NQH // 32], F32, tag=f"rsq{hi}")
                                nc.gpsimd.dma_start(
                                    rsq[:], rscrs[hi].rearrange("(a b) -> a b", a=32))
                                rsqs.append(rsq)
                            for hi in range(2):
                                rrec = pB.tile([32, NQH // 32], F32, tag=f"rrec{hi}")
                                nc.vector.reciprocal(rrec[:], rsqs[hi][:])
                                rrecs.append(rrec)
                            for hi in range(2):
                                rscr2 = dpool.tile([NQH], F32)
                                nc.gpsimd.dma_start(
                                    rscr2.rearrange("(a b) -> a b", a=32), rrecs[hi][:])
                                rscr2s.append(rscr2)
                            for hi in range(2):
                                rb = pB.tile([64, NQH], F32, tag=f"rb{hi}")
                                nc.gpsimd.dma_start(rb[:],
                                                    rscr2s[hi].partition_broadcast(64))
                                rbs.append(rb)
                            for hi in range(2):
                                h = 2 * hp + hi
                                base = 64 * (h % 2)
                                ft = h // 2
                                nc.vector.tensor_mul(
                                    ctxT[base:base + 64, ft, qoff:qoff + NQH],
                                    cus[hi][0:64, :], rbs[hi][:])

                # ---------------- Phase C: output projection ----------------
                with (
                    tc.tile_pool(name="pC", bufs=2) as pC,
                    tc.tile_pool(name="psC", bufs=2, space="PSUM") as ppsC,
                ):
                    for ot in range(D // P):
                        for tq in range(TT4):
                            tsl = slice(tq * NT, (tq + 1) * NT)
                            po = ppsC.tile([P, NT], F32, tag="po", name="po")
                            for fc in range(FT):
                                nc.tensor.matmul(po[:],
                                                 ow_sb[:, fc, ot * P:(ot + 1) * P],
                                                 ctxT[:, fc, tsl],
                                                 start=(fc == 0), stop=(fc == FT - 1))
                            o_sb = pC.tile([P, NT], F32, tag="oT")
                            nc.scalar.copy(o_sb[:], po[:])
                            nc.sync.dma_start(outT.ap()[ot * P:(ot + 1) * P, tsl],
                                              o_sb[:])

    nc.compile()
    return nc


def make_core_inputs(hidden_states, attention_bias, rope_cos, rope_sin, head_mask,
                     qkv_w, qkv_b, o_w, S=2048, D=1024, fast=True):
    """Host-side sharding + layout preparation. Returns list of 8 input dicts."""
    f32 = np.float32
    mt = np.float16 if fast else np.float32
    f16 = np.float16
    hidden_states = np.asarray(hidden_states, f32)
    attention_bias = np.asarray(attention_bias, f32)
    rope_cos = np.asarray(rope_cos, f32)
    rope_sin = np.asarray(rope_sin, f32)
    head_mask = np.asarray(head_mask, f32).reshape(-1)
    qkv_w = np.asarray(qkv_w, f32)
    qkv_b = np.asarray(qkv_b, f32)
    o_w = np.asarray(o_w, f32)

    B = hidden_states.shape[0]
    FPC = HPC * 64
    F = H * 64  # qkv feature dim (row-section size of qkv_w)

    def rot_rows(w):
        # rows indexed by f = hl*64 + d; rot(q)[d] = -q[d+32] (d<32) else q[d-32]
        w = w.reshape(HPC, 64, -1) if w.ndim == 2 else w.reshape(HPC, 64)
        lo, hi = w[:, 0:32], w[:, 32:64]
        out = np.concatenate([-hi, lo], axis=1)
        return out.reshape(HPC * 64, -1) if out.ndim == 3 else out.reshape(HPC * 64)

    cos_t = rope_cos[0, :, 0, :].T.astype(f32)     # [64, S]
    sin_t = rope_sin[0, :, 0, :].T.astype(f32)
    cosr = np.concatenate([cos_t, cos_t], axis=0)  # [128, S]
    sinr = np.concatenate([sin_t, sin_t], axis=0)

    in_maps = []
    for c in range(8):
        b, g = divmod(c, G)
        fs = slice(g * FPC, (g + 1) * FPC)
        wq = qkv_w[F * 0:F * 1][fs]
        wk = qkv_w[F * 1:F * 2][fs]
        wv = qkv_w[F * 2:F * 3][fs].copy()
        bq = qkv_b[F * 0:F * 1][fs]
        bk = qkv_b[F * 1:F * 2][fs]
        bvv = qkv_b[F * 2:F * 3][fs].copy()
        mask = head_mask[g * HPC:(g + 1) * HPC]
        wv *= np.repeat(mask, 64)[:, None]
        bvv *= np.repeat(mask, 64)
        wqr, bqr = rot_rows(wq), rot_rows(bq)
        wkr, bkr = rot_rows(wk), rot_rows(bk)
        w4 = np.concatenate([wq.T, wqr.T, wk.T, wkr.T], axis=1)  # [D, 4*FPC]
        b4 = np.concatenate([bq, bqr, bk, bkr])
        bT = np.ascontiguousarray(attention_bias[b, 0].T)
        m = {
            "hT": np.ascontiguousarray(hidden_states[b].T).astype(mt),
            "w4": np.ascontiguousarray(w4).astype(mt),
            "b4": np.ascontiguousarray(b4),
            "wvT": np.ascontiguousarray(wv.T).astype(mt),
            "bv": np.ascontiguousarray(bvv).astype(mt),
            "cosr": np.ascontiguousarray(cosr),
            "sinr": np.ascontiguousarray(sinr),
            "owT": np.ascontiguousarray(o_w[:, g * FPC:(g + 1) * FPC].T).astype(mt),
        }
        if fast:
            m["expbT"] = np.exp(bT).astype(f16)
        else:
            m["biasT"] = bT
        in_maps.append(m)
    return in_maps


def kernel(hidden_states, attention_bias, rope_cos, rope_sin, head_mask,
           qkv_w, qkv_b, o_w, o_b, **_unused):
    from concourse.bass_utils import run_bass_kernel_spmd

    B, S, D = hidden_states.shape
    fast = _CACHE.get("fast", True)
    if "nc" not in _CACHE:
        _CACHE["nc"] = build_nc(S=S, D=D, fast=fast)
    nc = _CACHE["nc"]

    in_maps = make_core_inputs(hidden_states, attention_bias, rope_cos, rope_sin,
                               head_mask, qkv_w, qkv_b, o_w, S=S, D=D, fast=fast)
    res = run_bass_kernel_spmd(nc, in_maps, list(range(8)))
    _CACHE["last_results"] = res

    o_b = np.asarray(o_b, np.float32)
    out = np.empty((B, S, D), np.float32)
    for b in range(B):
        acc = res.results[2 * b]["outT"].T + res.results[2 * b + 1]["outT"].T
        out[b] = acc + o_b[None, :]
    return out
